# revision 13
# baseline (speedup 1.0000x reference)
"""MoE layer (top-2 of 8 experts) on 8 Trainium2 NeuronCores.

Strategy: hidden-dimension-parallel with full token replication — every
core processes ALL dispatched (token, expert) pairs over its own 1/8
slice of the hidden dimension (512 of 4096 units). Per-core work is
exactly the mean load (sum of per-expert token counts / 8), so the SPMD
program is perfectly balanced regardless of routing skew, unlike
expert-per-core which pays for the most-loaded expert on every core.

Gating/top-k/softmax run on host (numpy) — they are ~0.003% of the
FLOPs. Host dispatches tokens into per-expert batches, cores compute
partial FFN outputs in bf16, host sums the 8 partial outputs, applies
the top-2 softmax combine weights, and adds the b2 term.

Device layout per core (static shapes, per-expert capacities padded to
a multiple of 16, identical across cores):
  phase 1: H^T[h,t] = relu(sum_k W1e[k, hslice]^T-chunk.T @ X^T[k,t]) —
           hidden lands on partitions so phase 2 needs no transpose.
  phase 2: Y[t,d] += (H^T chunk).T @ W2e[hslice, d]   (partial over h)

DMA ordering: the SP HWDGE ring drains dma_starts FIFO in emission
order, so loads are emitted in consumption order (xT of expert 0, first
W1 block, expert-0 W2 rows, then the rest expert by expert). Stores ride
the ACT ring so they never block loads.
"""

import math

import numpy as np
import ml_dtypes

D_MODEL = 1024
D_HIDDEN = 4096
NUM_EXPERTS = 8
TOP_K = 2

_KD = D_MODEL // 128     # contraction chunks in phase 1
_HSL = D_HIDDEN // 8     # per-core hidden slice (512)
_NHQ = _HSL // 128       # h-chunks per expert slice (4)
_TOK_TILE = 512

_compiled_cache: dict[tuple, object] = {}


def _ensure_paths():
    import sys
    for p in ("/opt/trn_rl_repo", "/opt/pypackages"):
        if p not in sys.path:
            sys.path.append(p)


def _tiles(cap):
    out, off = [], 0
    while off < cap:
        out.append((off, min(_TOK_TILE, cap - off)))
        off += _TOK_TILE
    return out


def _build(caps):
    """Compile the per-core program for per-expert capacities `caps`
    (each a multiple of 16; identical across cores)."""
    _ensure_paths()
    import concourse.bacc as bacc
    import concourse.mybir as mybir
    import concourse.tile as tile

    f32 = mybir.dt.float32
    bf16 = mybir.dt.bfloat16
    starts = [0]
    for c in caps:
        starts.append(starts[-1] + c)
    total = starts[-1]

    nc = bacc.Bacc("TRN2", target_bir_lowering=False, debug=False, num_devices=8)
    xT = nc.dram_tensor("xT", [D_MODEL, total], bf16, kind="ExternalInput")
    # w1s: per-core [D, 8*512] — cols e*512:(e+1)*512 = W1[e][:, hslice]
    w1 = nc.dram_tensor("w1s", [D_MODEL, D_HIDDEN], bf16, kind="ExternalInput")
    # w2s: per-core [8*512, D] — rows e*512:(e+1)*512 = W2[e][hslice, :]
    w2 = nc.dram_tensor("w2s", [D_HIDDEN, D_MODEL], bf16, kind="ExternalInput")
    # b1s: [128, 32] — col e*4+q = b1[e][hslice][q*128:(q+1)*128]
    b1c = nc.dram_tensor("b1s", [128, 32], f32, kind="ExternalInput")
    y = nc.dram_tensor("y", [total, D_MODEL], f32, kind="ExternalOutput")

    relu = mybir.ActivationFunctionType.Relu

    with tile.TileContext(nc) as tc:
        with (
            tc.tile_pool(name="wp", bufs=1) as wp,
            tc.tile_pool(name="xp", bufs=2) as xp,
            tc.tile_pool(name="hp", bufs=2) as hp,
            tc.tile_pool(name="yp", bufs=4) as yp,
            tc.tile_pool(name="ps1", bufs=4, space="PSUM") as ps1,
            tc.tile_pool(name="ps2", bufs=4, space="PSUM") as ps2,
        ):
            b1t = wp.tile([128, 32], f32, tag="b1", name="b1t")
            nc.sync.dma_start(b1t[:], b1c.ap())

            def load_xtile(ex, off, tsz):
                ts = [xp.tile([128, tsz], bf16, tag=f"x_{k}", name=f"xt{k}")
                      for k in range(_KD)]
                base = starts[ex] + off
                for k in range(_KD):
                    nc.sync.dma_start(
                        ts[k][:], xT.ap()[k * 128:(k + 1) * 128, base:base + tsz]
                    )
                return ts

            # w1 column blocks of 1024 (= two experts' slices each)
            w1t = [[wp.tile([128, 1024], bf16, tag=f"w1_{k}_{cb}",
                            name=f"w1t{k}_{cb}") for cb in range(4)]
                   for k in range(_KD)]

            def load_w1_block(cb):
                for k in range(_KD):
                    nc.sync.dma_start(
                        w1t[k][cb][:],
                        w1.ap()[k * 128:(k + 1) * 128, cb * 1024:(cb + 1) * 1024]
                    )

            # w2 row chunks: chunk e*4+q = rows of expert e's slice
            w2t = [wp.tile([128, D_MODEL], bf16, tag=f"w2_{c}", name=f"w2t{c}")
                   for c in range(32)]

            def load_w2_expert(ex):
                for q in range(_NHQ):
                    c = ex * _NHQ + q
                    nc.sync.dma_start(w2t[c][:], w2.ap()[c * 128:(c + 1) * 128, :])

            # (expert, tile) work list in execution order
            work = [(ex, off, tsz) for ex in range(NUM_EXPERTS)
                    for (off, tsz) in _tiles(caps[ex])]

            # Emission order = HBM arrival order: feed the PE just-in-time,
            # expert by expert.
            xq = {}
            for ex in range(NUM_EXPERTS):
                if ex % 2 == 0:
                    load_w1_block(ex // 2)
                for i, (wex, off, tsz) in enumerate(work):
                    if wex == ex:
                        xq[i] = load_xtile(wex, off, tsz)
                load_w2_expert(ex)

            for i, (ex, off, tsz) in enumerate(work):
                xt_cur = xq[i]
                h_t = [hp.tile([128, tsz], bf16, tag=f"h_{q}", name=f"ht{q}")
                       for q in range(_NHQ)]
                cb, sub = divmod(ex, 2)
                for q in range(_NHQ):
                    acc = ps1.tile([128, tsz], f32, name="acc1")
                    for k in range(_KD):
                        nc.tensor.matmul(
                            acc[:],
                            w1t[k][cb][:, sub * _HSL + q * 128:
                                       sub * _HSL + (q + 1) * 128],
                            xt_cur[k][:],
                            start=(k == 0),
                            stop=(k == _KD - 1),
                        )
                    nc.scalar.activation(
                        h_t[q][:], acc[:], relu,
                        bias=b1t[:, ex * _NHQ + q:ex * _NHQ + q + 1]
                    )
                for m0 in range(0, tsz, 128):
                    msz = min(128, tsz - m0)
                    for d0 in range(0, D_MODEL, 512):
                        acc2 = ps2.tile([128, 512], f32, name="acc2")
                        for q in range(_NHQ):
                            nc.tensor.matmul(
                                acc2[:msz],
                                h_t[q][:, m0:m0 + msz],
                                w2t[ex * _NHQ + q][:, d0:d0 + 512],
                                start=(q == 0),
                                stop=(q == _NHQ - 1),
                            )
                        yo = yp.tile([128, 512], f32, name="yo")
                        nc.vector.tensor_copy(yo[:msz], acc2[:msz])
                        base = starts[ex] + off
                        # stores ride the ACT HWDGE ring so the SP ring stays
                        # a pure in-order load pipe
                        nc.scalar.dma_start(
                            y.ap()[base + m0:base + m0 + msz, d0:d0 + 512],
                            yo[:msz]
                        )

    nc.compile()
    return nc


def kernel(x, Wg, bg, W1, b1, W2, b2):
    _ensure_paths()
    from concourse.bass_utils import run_bass_kernel_spmd

    x = np.asarray(x, np.float32)
    Wg = np.asarray(Wg, np.float32)
    bg = np.asarray(bg, np.float32)
    W1 = np.asarray(W1, np.float32)
    b1 = np.asarray(b1, np.float32)
    W2 = np.asarray(W2, np.float32)
    b2 = np.asarray(b2, np.float32)

    B, S, D = x.shape
    xt = x.reshape(-1, D)
    T = xt.shape[0]

    # --- Gating on host (mirrors jax.lax.top_k: descending, stable) ---
    logits = xt @ Wg + bg
    order = np.argsort(-logits, axis=-1, kind="stable")
    idx = order[:, :TOP_K]                                  # [T, K]
    vals = np.take_along_axis(logits, idx, axis=1)          # [T, K] desc
    e = np.exp(vals - vals[:, :1])
    w = (e / e.sum(axis=1, keepdims=True)).astype(np.float32)  # [T, K]

    # --- Dispatch ---
    ids_per_e = [np.nonzero((idx == ex).any(axis=1))[0] for ex in range(NUM_EXPERTS)]
    caps = tuple(16 * math.ceil(max(len(ids), 1) / 16) for ids in ids_per_e)
    starts = [0]
    for c in caps:
        starts.append(starts[-1] + c)
    total = starts[-1]

    nc = _compiled_cache.get(caps)
    if nc is None:
        nc = _compiled_cache[caps] = _build(caps)

    xTall = np.zeros((D_MODEL, total), ml_dtypes.bfloat16)
    for ex in range(NUM_EXPERTS):
        ids = ids_per_e[ex]
        xTall[:, starts[ex]:starts[ex] + len(ids)] = \
            xt[ids].astype(ml_dtypes.bfloat16).T

    W1b = W1.astype(ml_dtypes.bfloat16)   # [E, D, H]
    W2b = W2.astype(ml_dtypes.bfloat16)   # [E, H, D]
    in_maps = []
    for c in range(8):
        hs = slice(c * _HSL, (c + 1) * _HSL)
        w1s = np.concatenate([W1b[ex][:, hs] for ex in range(NUM_EXPERTS)], axis=1)
        w2s = np.concatenate([W2b[ex][hs, :] for ex in range(NUM_EXPERTS)], axis=0)
        b1s = np.ascontiguousarray(
            b1[:, hs].reshape(NUM_EXPERTS * _NHQ, 128).T)      # [128, 32]
        in_maps.append({
            "xT": xTall,
            "w1s": np.ascontiguousarray(w1s),
            "w2s": np.ascontiguousarray(w2s),
            "b1s": b1s,
        })

    res = run_bass_kernel_spmd(nc, in_maps, core_ids=list(range(8)))

    # --- Combine on host: sum the 8 hidden-slice partials, then the
    # top-2 softmax-weighted scatter, then the b2 term. ---
    ysum = np.asarray(res.results[0]["y"], np.float32).copy()
    for c in range(1, 8):
        ysum += np.asarray(res.results[c]["y"], np.float32)

    out = np.zeros((T, D_MODEL), np.float32)
    for ex in range(NUM_EXPERTS):
        ids = ids_per_e[ex]
        if len(ids) == 0:
            continue
        ye = ysum[starts[ex]:starts[ex] + len(ids)]
        pos = (idx[ids] == ex).argmax(axis=1)
        ce = np.take_along_axis(w[ids], pos[:, None], axis=1)[:, 0]
        out[ids] += ye * ce[:, None]
    out += np.einsum("tk,tkd->td", w, b2[idx])

    return out.reshape(B, S, D), logits


# revision 14
# speedup vs baseline: 1.1575x; 1.1575x over previous
"""MoE layer (top-2 of 8 experts) on 8 Trainium2 NeuronCores.

Strategy: expert-pair parallel. Experts are paired (largest token count
with smallest) and each pair is assigned two cores; each core computes
BOTH experts of its pair over half the hidden dimension (2048 of 4096).
Per-core work is the pair-average token count, which smooths routing
skew at no extra HBM cost (weights split, not replicated; activations
replicated only within the pair).

Gating/top-k/softmax run on host (numpy) — they are ~0.003% of the
FLOPs. Host dispatches tokens into per-expert batches, cores compute
partial FFN outputs in bf16, host sums each pair's two half-H partials,
applies the top-2 softmax combine weights, and adds the b2 term.

Device layout per core (static shapes, capacities padded to a multiple
of 16, identical across cores):
  phase 1: H^T[h,t] = relu(sum_k W1e[k, hslice]^T-chunk.T @ X^T[k,t]) —
           hidden lands on partitions so phase 2 needs no transpose.
  phase 2: Y[t,d] += (H^T chunk).T @ W2e[hslice, d]   (partial over h)

DMA ordering: the SP HWDGE ring drains dma_starts FIFO in emission
order, so loads are emitted in consumption order (expert A's first
tokens and first W1 block gate the start; everything else streams in
behind). Stores ride the ACT ring so they never block loads.
"""

import math

import numpy as np
import ml_dtypes

D_MODEL = 1024
D_HIDDEN = 4096
NUM_EXPERTS = 8
TOP_K = 2

_KD = D_MODEL // 128     # contraction chunks in phase 1 (8)
_HSL = D_HIDDEN // 2     # per-core hidden slice (2048)
_NHQ = _HSL // 128       # h-chunks per expert slice (16)
_TOK_TILE = 512

_compiled_cache: dict[tuple, object] = {}


def _ensure_paths():
    import sys
    for p in ("/opt/trn_rl_repo", "/opt/pypackages"):
        if p not in sys.path:
            sys.path.append(p)


def _tiles(cap):
    out, off = [], 0
    while off < cap:
        out.append((off, min(_TOK_TILE, cap - off)))
        off += _TOK_TILE
    return out


def _build(caps):
    """Compile the per-core program for (capA, capB) token capacities.

    Per-core DRAM views (prepared host-side):
      xT  [1024, capA+capB]  bf16 — expert A's tokens then expert B's
      w1s [1024, 2*2048]     bf16 — cols 0:2048 = W1[A][:, hslice],
                                    cols 2048:  = W1[B][:, hslice]
      w2s [2*2048, 1024]     bf16 — rows 0:2048 = W2[A][hslice, :], ...
      b1s [128, 32]          f32  — col e*16+q = b1 chunk q of expert e
      y   [capA+capB, 1024]  f32  — half-H partial outputs
    """
    _ensure_paths()
    import concourse.bacc as bacc
    import concourse.mybir as mybir
    import concourse.tile as tile

    f32 = mybir.dt.float32
    bf16 = mybir.dt.bfloat16
    total = sum(caps)
    starts = [0, caps[0]]

    nc = bacc.Bacc("TRN2", target_bir_lowering=False, debug=False, num_devices=8)
    xT = nc.dram_tensor("xT", [D_MODEL, total], bf16, kind="ExternalInput")
    w1 = nc.dram_tensor("w1s", [D_MODEL, 2 * _HSL], bf16, kind="ExternalInput")
    w2 = nc.dram_tensor("w2s", [2 * _HSL, D_MODEL], bf16, kind="ExternalInput")
    b1c = nc.dram_tensor("b1s", [128, 32], f32, kind="ExternalInput")
    y = nc.dram_tensor("y", [total, D_MODEL], f32, kind="ExternalOutput")

    relu = mybir.ActivationFunctionType.Relu

    with tile.TileContext(nc) as tc:
        with (
            tc.tile_pool(name="wp", bufs=1) as wp,
            tc.tile_pool(name="xp", bufs=2) as xp,
            tc.tile_pool(name="hp", bufs=1) as hp,
            tc.tile_pool(name="yp", bufs=4) as yp,
            tc.tile_pool(name="ps1", bufs=4, space="PSUM") as ps1,
            tc.tile_pool(name="ps2", bufs=4, space="PSUM") as ps2,
        ):
            b1t = wp.tile([128, 32], f32, tag="b1", name="b1t")
            nc.sync.dma_start(b1t[:], b1c.ap())

            def load_xtile(ex, off, tsz):
                ts = [xp.tile([128, tsz], bf16, tag=f"x_{k}", name=f"xt{k}")
                      for k in range(_KD)]
                base = starts[ex] + off
                for k in range(_KD):
                    nc.sync.dma_start(
                        ts[k][:], xT.ap()[k * 128:(k + 1) * 128, base:base + tsz]
                    )
                return ts

            # w1 as 4 column blocks of 1024 (blocks 0-1 expert A, 2-3 B)
            w1t = [[wp.tile([128, 1024], bf16, tag=f"w1_{k}_{cb}",
                            name=f"w1t{k}_{cb}") for cb in range(4)]
                   for k in range(_KD)]

            def load_w1_block(cb):
                for k in range(_KD):
                    nc.sync.dma_start(
                        w1t[k][cb][:],
                        w1.ap()[k * 128:(k + 1) * 128, cb * 1024:(cb + 1) * 1024]
                    )

            # w2 row chunks: chunk ex*16+q
            w2t = [wp.tile([128, D_MODEL], bf16, tag=f"w2_{c}", name=f"w2t{c}")
                   for c in range(2 * _NHQ)]

            def load_w2_expert(ex):
                for q in range(_NHQ):
                    c = ex * _NHQ + q
                    nc.sync.dma_start(w2t[c][:], w2.ap()[c * 128:(c + 1) * 128, :])

            work = [(ex, off, tsz) for ex in range(2)
                    for (off, tsz) in _tiles(caps[ex])]

            # Emission order = HBM arrival order (SP ring is FIFO):
            # expert A: first tokens, W1 blocks, W2 rows, rest of tokens;
            # then the same for expert B.
            xq = {}
            a_tiles = [i for i, (ex, _, _) in enumerate(work) if ex == 0]
            b_tiles = [i for i, (ex, _, _) in enumerate(work) if ex == 1]
            xq[a_tiles[0]] = load_xtile(*work[a_tiles[0]])
            load_w1_block(0)
            load_w1_block(1)
            for i in a_tiles[1:]:
                xq[i] = load_xtile(*work[i])
            load_w2_expert(0)
            xq[b_tiles[0]] = load_xtile(*work[b_tiles[0]])
            load_w1_block(2)
            load_w1_block(3)
            for i in b_tiles[1:]:
                xq[i] = load_xtile(*work[i])
            load_w2_expert(1)

            for i, (ex, off, tsz) in enumerate(work):
                xt_cur = xq[i]
                h_t = [hp.tile([128, tsz], bf16, tag=f"h_{q}", name=f"ht{q}")
                       for q in range(_NHQ)]
                for q in range(_NHQ):
                    cb = ex * 2 + q // 8
                    hc = q % 8
                    acc = ps1.tile([128, tsz], f32, name="acc1")
                    for k in range(_KD):
                        nc.tensor.matmul(
                            acc[:],
                            w1t[k][cb][:, hc * 128:(hc + 1) * 128],
                            xt_cur[k][:],
                            start=(k == 0),
                            stop=(k == _KD - 1),
                        )
                    nc.scalar.activation(
                        h_t[q][:], acc[:], relu,
                        bias=b1t[:, ex * _NHQ + q:ex * _NHQ + q + 1]
                    )
                for m0 in range(0, tsz, 128):
                    msz = min(128, tsz - m0)
                    for d0 in range(0, D_MODEL, 512):
                        acc2 = ps2.tile([128, 512], f32, name="acc2")
                        for q in range(_NHQ):
                            nc.tensor.matmul(
                                acc2[:msz],
                                h_t[q][:, m0:m0 + msz],
                                w2t[ex * _NHQ + q][:, d0:d0 + 512],
                                start=(q == 0),
                                stop=(q == _NHQ - 1),
                            )
                        yo = yp.tile([128, 512], f32, name="yo")
                        nc.vector.tensor_copy(yo[:msz], acc2[:msz])
                        base = starts[ex] + off
                        # stores ride the ACT HWDGE ring so the SP ring
                        # stays a pure in-order load pipe
                        nc.scalar.dma_start(
                            y.ap()[base + m0:base + m0 + msz, d0:d0 + 512],
                            yo[:msz]
                        )

    nc.compile()
    return nc


def kernel(x, Wg, bg, W1, b1, W2, b2):
    _ensure_paths()
    from concourse.bass_utils import run_bass_kernel_spmd

    x = np.asarray(x, np.float32)
    Wg = np.asarray(Wg, np.float32)
    bg = np.asarray(bg, np.float32)
    W1 = np.asarray(W1, np.float32)
    b1 = np.asarray(b1, np.float32)
    W2 = np.asarray(W2, np.float32)
    b2 = np.asarray(b2, np.float32)

    B, S, D = x.shape
    xt = x.reshape(-1, D)
    T = xt.shape[0]

    # --- Gating on host (mirrors jax.lax.top_k: descending, stable) ---
    logits = xt @ Wg + bg
    order = np.argsort(-logits, axis=-1, kind="stable")
    idx = order[:, :TOP_K]                                  # [T, K]
    vals = np.take_along_axis(logits, idx, axis=1)          # [T, K] desc
    e = np.exp(vals - vals[:, :1])
    w = (e / e.sum(axis=1, keepdims=True)).astype(np.float32)  # [T, K]

    # --- Dispatch: pair largest-count expert with smallest ---
    ids_per_e = [np.nonzero((idx == ex).any(axis=1))[0] for ex in range(NUM_EXPERTS)]
    counts = [len(ids) for ids in ids_per_e]
    by_count = sorted(range(NUM_EXPERTS), key=lambda ex: -counts[ex])
    pairs = [(by_count[p], by_count[NUM_EXPERTS - 1 - p]) for p in range(4)]
    capA = 16 * math.ceil(max(max(counts[a] for a, _ in pairs), 1) / 16)
    capB = 16 * math.ceil(max(max(counts[b] for _, b in pairs), 1) / 16)
    caps = (capA, capB)

    nc = _compiled_cache.get(caps)
    if nc is None:
        nc = _compiled_cache[caps] = _build(caps)

    W1b = W1.astype(ml_dtypes.bfloat16)   # [E, D, H]
    W2b = W2.astype(ml_dtypes.bfloat16)   # [E, H, D]
    in_maps = []
    for c in range(8):
        a, b = pairs[c // 2]
        hs = slice((c % 2) * _HSL, (c % 2 + 1) * _HSL)
        xTe = np.zeros((D_MODEL, capA + capB), ml_dtypes.bfloat16)
        xTe[:, :counts[a]] = xt[ids_per_e[a]].astype(ml_dtypes.bfloat16).T
        xTe[:, capA:capA + counts[b]] = \
            xt[ids_per_e[b]].astype(ml_dtypes.bfloat16).T
        w1s = np.concatenate([W1b[a][:, hs], W1b[b][:, hs]], axis=1)
        w2s = np.concatenate([W2b[a][hs, :], W2b[b][hs, :]], axis=0)
        b1s = np.ascontiguousarray(
            np.concatenate([b1[a][hs], b1[b][hs]]).reshape(32, 128).T)
        in_maps.append({
            "xT": xTe,
            "w1s": np.ascontiguousarray(w1s),
            "w2s": np.ascontiguousarray(w2s),
            "b1s": b1s,
        })

    res = run_bass_kernel_spmd(nc, in_maps, core_ids=list(range(8)))

    # --- Combine on host ---
    out = np.zeros((T, D_MODEL), np.float32)
    for p, (a, b) in enumerate(pairs):
        ysum = np.asarray(res.results[2 * p]["y"], np.float32) + \
            np.asarray(res.results[2 * p + 1]["y"], np.float32)
        for ex, s in ((a, 0), (b, capA)):
            ids = ids_per_e[ex]
            if len(ids) == 0:
                continue
            ye = ysum[s:s + len(ids)]
            pos = (idx[ids] == ex).argmax(axis=1)
            ce = np.take_along_axis(w[ids], pos[:, None], axis=1)[:, 0]
            out[ids] += ye * ce[:, None]
    out += np.einsum("tk,tkd->td", w, b2[idx])

    return out.reshape(B, S, D), logits
